# revision 1
# baseline (speedup 1.0000x reference)
"""Self-contained TRN2 Bass kernel for NeuralFSM message passing (v2).

kernel(s0, edge_index, T) -> [100000, 8] float32: 20 FSM iterations on 8
NeuronCores via concourse/bass (SPMD).

Algorithm: per iteration, each edge must deliver byte 1<<state[src] to an
OR-accumulator at its destination. Instead of per-edge ap_gather (27ns/idx,
latency-bound), edges are routed with hardware-scatter primitives:
  expand (DVE broadcast) -> local_scatter into per-dst-partition buckets ->
  one XBAR DMA transpose (cross-partition routing) -> masked local_scatter
  into dst-slot runs -> DVE OR-reduce -> mask AllToAll across cores ->
  FSM lookup via one small ap_gather -> byte AllGather -> table rebuild via
  local_scatter.
Edge->core and edge->block assignments use vectorized Euler splits so every
per-(node,core) and per-(bucket) count is balanced within +-1, keeping all
padding near-optimal.
"""
import os
import sys

import numpy as np

for _p in ("/opt/trn_rl_repo", "/root/.axon_site/_ro/trn_rl_repo", "/root/.axon_site"):
    if os.path.isdir(_p) and _p not in sys.path:
        sys.path.append(_p)


N_REAL = 100000
S = 8
NC = 8
P = 128
NPP = 784
NPC = 98
NTOT = NC * P * NPC  # 100352
ITERS = 20
G = 8
RSUB = 2            # uniform expand sub-run (max node count per (core, block) is 2)


def _cumcount(keys):
    n = len(keys)
    if n == 0:
        return np.zeros(0, np.int64)
    run_start = np.r_[True, keys[1:] != keys[:-1]]
    idx = np.arange(n)
    return idx - np.maximum.accumulate(np.where(run_start, idx, 0))


def _pair_within_runs(order, keys_sorted, size):
    m = np.arange(size)
    pos = _cumcount(keys_sorted)
    cont = np.r_[keys_sorted[1:] == keys_sorted[:-1], False]
    first = (pos % 2 == 0) & cont
    i = np.where(first)[0]
    a, b = order[i], order[i + 1]
    m[a] = b
    m[b] = a
    return m


def euler_color(src, dst):
    """2-color edges: per-src and per-dst counts split into floor/ceil halves."""
    E = len(src)
    sdeg = np.bincount(src)
    ddeg = np.bincount(dst)
    odd_s = np.where(sdeg % 2 == 1)[0]
    odd_d = np.where(ddeg % 2 == 1)[0]
    VS = int(src.max()) + 1
    VD = int(dst.max()) + 1
    ex_src = np.concatenate([src, odd_s, np.full(len(odd_d), VS, np.int64)])
    ex_dst = np.concatenate([dst, np.full(len(odd_s), VD, np.int64), odd_d])
    if len(odd_s) % 2 == 1:
        ex_src = np.r_[ex_src, VS]
        ex_dst = np.r_[ex_dst, VD]
    Etot = len(ex_src)
    o1 = np.argsort(ex_src, kind="stable")
    m1 = _pair_within_runs(o1, ex_src[o1], Etot)
    o2 = np.argsort(ex_dst, kind="stable")
    m2 = _pair_within_runs(o2, ex_dst[o2], Etot)
    g = m2[m1]
    r = np.arange(Etot)
    gg = g.copy()
    for _ in range(30):
        r2 = np.minimum(r, r[gg])
        gg = gg[gg]
        if np.array_equal(r2, r):
            r2 = np.minimum(r2, r2[gg])
            if np.array_equal(r2, r):
                break
        r = r2
    r = np.minimum(r, r[g])
    return (r > r[m1]).astype(np.int8)[:E]


def euler_split8(src, dst):
    E = len(src)
    lab = euler_color(src, dst).astype(np.int64)
    for level in range(1, 3):
        new = np.zeros(E, np.int64)
        for part in range(1 << level):
            sel = np.where(lab == part)[0]
            if len(sel):
                c = euler_color(src[sel], dst[sel])
                new[sel] = (part << 1) | c
        lab = new
    return lab


def _row_classes(vals):
    out = []
    i, n = 0, len(vals)
    while i < n:
        j = i
        while j < n and vals[j] == vals[i]:
            j += 1
        out.append((i, j - i, int(vals[i])))
        i = j
    return out


class Layout2:
    def __init__(self, edge_index):
        src = edge_index[0].astype(np.int64)
        dst = edge_index[1].astype(np.int64)
        E = len(src)

        deg_out = np.bincount(src, minlength=NTOT)
        deg_in = np.bincount(dst, minlength=NTOT)

        order = np.argsort(-deg_out, kind="stable")
        part_of_node = np.empty(NTOT, np.int64)
        part_of_node[order] = np.arange(NTOT) % P

        self.k_src = np.empty(NTOT, np.int64)
        self.k_dst = np.empty(NTOT, np.int64)
        self.node_at_src = np.empty((P, NPP), np.int64)
        self.node_at_dst = np.empty((P, NPP), np.int64)
        for p in range(P):
            nodes = np.where(part_of_node == p)[0]
            o = nodes[np.argsort(-deg_out[nodes], kind="stable")]
            self.k_src[o] = np.arange(NPP)
            self.node_at_src[p] = o
            o2 = nodes[np.argsort(-deg_in[nodes], kind="stable")]
            self.k_dst[o2] = np.arange(NPP)
            self.node_at_dst[p] = o2
        self.part_of_node = part_of_node

        # core split
        ecore = euler_split8(src, dst)
        self.ecore = ecore
        p_src = part_of_node[src]
        p_dst = part_of_node[dst]
        ks = self.k_src[src]
        kd = self.k_dst[dst]

        # block split per (core, p_src): bipartite (node, q)
        lv = (ecore * P + p_src) * NPP + ks        # left vertex id
        rv = (ecore * P + p_src) * P + p_dst       # right vertex id
        eblock = euler_split8(lv, rv)
        self.eblock = eblock

        # expand stream: G blocks x NPP rows x RSUB slots
        # position of edge: block*NPP*RSUB + k_src*RSUB + pos_in (node,core,block) run
        cntg = np.zeros((NC, P, NPP, G), np.int32)
        np.add.at(cntg, (ecore, p_src, ks, eblock), 1)
        assert cntg.max() <= RSUB, cntg.max()
        self.S_exp = G * NPP * RSUB               # 12544

        # reduce runs: L_pad[k] = max over (c,p) of in-count, >=1
        cin = np.zeros((NC, P, NPP), np.int32)
        np.add.at(cin, (ecore, p_dst, kd), 1)
        self.L_pad = np.maximum(1, cin.max(axis=(0, 1))).astype(np.int64)
        self.red_off = np.concatenate([[0], np.cumsum(self.L_pad)])
        self.S_red = int(self.red_off[-1])

        # V windows over k_dst (cumulative L_pad <= 2046)
        self.v_bounds = [0]
        cur = 0
        for k in range(NPP):
            if cur + self.L_pad[k] > 2046:
                self.v_bounds.append(k)
                cur = 0
            cur += self.L_pad[k]
        self.v_bounds.append(NPP)
        self.V = len(self.v_bounds) - 1
        self.v_of_k = np.zeros(NPP, np.int64)
        for v in range(self.V):
            self.v_of_k[self.v_bounds[v]:self.v_bounds[v + 1]] = v
        self.W_v = [int(self.red_off[self.v_bounds[v + 1]] - self.red_off[self.v_bounds[v]])
                    for v in range(self.V)]
        self.W_v_pad = [w + (w % 2) for w in self.W_v]
        self.soff = np.concatenate([[0], np.cumsum(self.W_v_pad)]).astype(np.int64)

        # bucket counts -> B_g
        nb = np.zeros((NC, P, G, P), np.int32)
        np.add.at(nb, (ecore, p_src, eblock, p_dst), 1)
        self.B_g = nb.max(axis=(0, 1, 3)).astype(np.int64)   # [G]
        assert (128 * self.B_g <= 2046).all(), self.B_g
        self.colbase = np.concatenate([[0], np.cumsum(self.B_g)])
        self.C = int(self.colbase[-1])

        # ---- per-edge placement (all cores at once, vectorized) ----
        # expand pos within sub-run
        key_exp = (((ecore * P + p_src) * G + eblock) * NPP + ks)
        o = np.argsort(key_exp, kind="stable")
        sub = _cumcount(key_exp[o])
        t_exp = np.empty(E, np.int64)
        t_exp[o] = (eblock[o] * NPP + ks[o]) * RSUB + sub     # per-partition stream pos

        # R1 bucket fill -> col
        key_b = (((ecore * P + p_src) * G + eblock) * P + p_dst)
        o2 = np.lexsort((t_exp, key_b))
        fill = np.empty(E, np.int64)
        fill[o2] = _cumcount(key_b[o2])
        col = self.colbase[eblock] + fill                     # [E]

        # R3 slot fill
        key_d = (ecore * P + p_dst) * NPP + kd
        o3 = np.lexsort((t_exp, key_d))
        dfill = np.empty(E, np.int64)
        dfill[o3] = _cumcount(key_d[o3])
        slot = self.red_off[kd] + dfill                       # absolute (unpadded-window) slot
        # window-padded absolute slot offset:
        v_e = self.v_of_k[kd]
        wadj = self.soff[v_e] - self.red_off[np.array(self.v_bounds[:-1])][v_e]
        slot_pad = slot + wadj

        # ---- static tensors per core ----
        self.r1_idx = np.full((NC, P, self.S_exp), -1, np.int16)
        seg_len = NPP * RSUB                                  # 1568 per block
        win_rel = (col - self.colbase[eblock]) * P + p_dst
        rel_t = t_exp - eblock * seg_len
        self.r1_idx[ecore, p_src, t_exp] = win_rel.astype(np.int16)

        self.r3_idx = np.full((NC, self.V, P, self.C * P), -1, np.int16)
        tpos = col * P + p_src
        slot_rel = slot_pad - self.soff[v_e]
        self.r3_idx[ecore, v_e, p_dst, tpos] = slot_rel.astype(np.int16)

        # rebuild scatter idx
        self.rebuild_idx = np.empty((P, NPP), np.int16)
        for p in range(P):
            self.rebuild_idx[p] = self.k_src[self.node_at_dst[p]].astype(np.int16)

        # reduce row classes, split at v boundaries, with padded-window offsets
        self.red_classes = []   # (slot_off, k0, ln, L)
        for v in range(self.V):
            k0, k1 = self.v_bounds[v], self.v_bounds[v + 1]
            for (i, ln, L) in _row_classes(self.L_pad[k0:k1]):
                kk = k0 + i
                off = int(self.soff[v] + self.red_off[kk] - self.red_off[k0])
                self.red_classes.append((off, kk, ln, L))

        self.seg_len = seg_len
        self.S_slots = int(self.soff[-1])

    # ---- device input builders ----
    def build_table(self, state):
        tab = np.zeros((P, NPP), np.uint16)
        for p in range(P):
            tab[p] = (1 << state[self.node_at_src[p]]).astype(np.uint16)
        return tab

    def build_q0(self, state):
        q = np.zeros((NC, P, NPC), np.uint32)
        for c in range(NC):
            for p in range(P):
                q[c, p] = state[self.node_at_dst[p, c * NPC:(c + 1) * NPC]]
        return q

    def build_t2(self, T):
        NS = np.argmax(T, axis=2).astype(np.uint32)   # [256, 8]
        tab = np.zeros(2048, np.uint32)
        m = np.repeat(np.arange(256), 8)
        s = np.tile(np.arange(8), 256)
        tab[m * 8 + s] = (np.uint32(1) << NS[m, s]) << 16 | NS[m, s]
        return np.broadcast_to(tab, (P, 2048)).copy()

    def decode(self, qout):
        """qout: [NC][P, NPC] u32 -> one-hot [N_REAL, 8]."""
        st = np.zeros(NTOT, np.int64)
        for c in range(NC):
            for p in range(P):
                st[self.node_at_dst[p, c * NPC:(c + 1) * NPC]] = qout[c][p]
        out = np.zeros((N_REAL, S), np.float32)
        out[np.arange(N_REAL), st[:N_REAL]] = 1.0
        return out



LAST_EXEC_NS = None


def _build_kernel(lay, iters=ITERS):
    from concourse import bacc, tile, mybir

    u16 = mybir.dt.uint16
    u32 = mybir.dt.uint32
    i16 = mybir.dt.int16
    Alu = mybir.AluOpType
    X = mybir.AxisListType.X

    C = lay.C
    CF = C * P                   # transposed stream length per partition
    SEG = lay.seg_len            # 1568
    T2N = 16 * NPC               # 1568 t2 gather idxs per group

    nc = bacc.Bacc("TRN2", target_bir_lowering=False, debug=False,
                   enable_asserts=True, num_devices=NC)
    t_table0 = nc.dram_tensor("t_table0", [P, NPP], u16, kind="ExternalInput")
    t_q0 = nc.dram_tensor("t_q0", [P, NPC], u16, kind="ExternalInput")
    t_r1idx = nc.dram_tensor("t_r1idx", [P, lay.S_exp], i16, kind="ExternalInput")
    t_r3idx = nc.dram_tensor("t_r3idx", [P, lay.V * CF], i16, kind="ExternalInput")
    t_rebuild = nc.dram_tensor("t_rebuild", [P, NPP], i16, kind="ExternalInput")
    t_t2 = nc.dram_tensor("t_t2", [P, 2048], u32, kind="ExternalInput")
    t_m16 = nc.dram_tensor("t_m16", [P, 16], u32, kind="ExternalInput")
    t_qout = nc.dram_tensor("t_qout", [P, NPC], u16, kind="ExternalOutput")

    with tile.TileContext(nc) as tc:
        with tc.tile_pool(name="dram", bufs=2, space="DRAM") as dram, \
             tc.tile_pool(name="per", bufs=1) as per, \
             tc.tile_pool(name="r3p", bufs=3) as r3p, \
             tc.tile_pool(name="qq", bufs=2) as qq:
            table = per.tile([P, NPP], u16)
            r1idx = per.tile([P, lay.S_exp], i16)
            rebuild = per.tile([P, NPP], i16)
            t2tab = per.tile([P, 2048], u32)
            m16 = per.tile([P, 16], u32)
            stream = per.tile([P, lay.S_exp], u16)
            r1out = per.tile([P, CF], u16)
            trout = per.tile([P, CF], u16)
            slots = per.tile([P, lay.S_slots], u16)
            maskp = per.tile([P, NPP], u16)
            mask_rx = per.tile([P, NC * NPC], u16)
            maskf = per.tile([P, NPC], u16)
            idxu = per.tile([P, NPC], u16)
            t2tmp = per.tile([P, NPC], u32)
            idx16 = per.tile([P, NPC], i16)
            t2out = per.tile([P, T2N], u32)
            t2sel = per.tile([P, NPC], u32)
            bytes16 = per.tile([P, NPC], u16)
            data784 = per.tile([P, NPP], u16)

            nc.sync.dma_start(out=table[:], in_=t_table0[:])
            nc.sync.dma_start(out=r1idx[:], in_=t_r1idx[:])
            nc.sync.dma_start(out=rebuild[:], in_=t_rebuild[:])
            nc.sync.dma_start(out=t2tab[:], in_=t_t2[:])
            nc.sync.dma_start(out=m16[:], in_=t_m16[:])
            q = qq.tile([P, NPC], u16, tag="q")
            nc.sync.dma_start(out=q[:], in_=t_q0[:])

            for it in range(iters):
                # expand: stream[p, (g k r)] = table[p, k]
                for g in range(G):
                    nc.vector.tensor_copy(
                        stream[:, g * SEG:(g + 1) * SEG].rearrange(
                            "p (k r) -> p k r", r=RSUB),
                        table[:, :, None].broadcast_to([P, NPP, RSUB]))
                # R1 + per-window transpose (overlapped)
                for g in range(G):
                    ne = 128 * int(lay.B_g[g])
                    base = lay.colbase[g] * P
                    nc.gpsimd.local_scatter(
                        out_ap=r1out[:, base:base + ne],
                        data_ap=stream[:, g * SEG:(g + 1) * SEG],
                        idxs_ap=r1idx[:, g * SEG:(g + 1) * SEG],
                        channels=P, num_elems=ne, num_idxs=SEG)
                    nc.sync.dma_start(
                        out=trout[:, base:base + ne].rearrange(
                            "p (b q) -> p b q", q=P),
                        in_=r1out[:, base:base + ne], transpose=True)
                # R3 (idx streamed from DRAM, double-buffered)
                for v in range(lay.V):
                    r3i = r3p.tile([P, CF], i16, tag="r3i")
                    nc.sync.dma_start(out=r3i[:], in_=t_r3idx[:, v * CF:(v + 1) * CF])
                    nc.gpsimd.local_scatter(
                        out_ap=slots[:, int(lay.soff[v]):int(lay.soff[v]) + lay.W_v_pad[v]],
                        data_ap=trout[:],
                        idxs_ap=r3i[:],
                        channels=P, num_elems=lay.W_v_pad[v], num_idxs=CF)
                # reduce
                for (off, k0, ln, L) in lay.red_classes:
                    if L == 1:
                        nc.vector.tensor_copy(maskp[:, k0:k0 + ln],
                                              slots[:, off:off + ln])
                    else:
                        nc.vector.tensor_reduce(
                            out=maskp[:, k0:k0 + ln],
                            in_=slots[:, off:off + ln * L].rearrange(
                                "p (a b) -> p a b", b=L),
                            axis=X, op=Alu.bitwise_or)
                # mask exchange (AllToAll)
                a2a_in = dram.tile([1, NTOT], u16, tag="a2ai")
                a2a_out = dram.tile([1, NTOT], u16, tag="a2ao")
                nc.sync.dma_start(
                    out=a2a_in[0:1, :].rearrange("x (c p j) -> (x p) c j", c=NC, p=P),
                    in_=maskp[:].rearrange("p (c j) -> p c j", c=NC))
                nc.gpsimd.collective_compute(
                    "AllToAll", Alu.bypass,
                    replica_groups=[list(range(NC))],
                    ins=[a2a_in.opt()], outs=[a2a_out.opt()])
                nc.sync.dma_start(
                    out=mask_rx[:].rearrange("p (c j) -> p c j", c=NC),
                    in_=a2a_out[0:1, :].rearrange("x (c p j) -> (x p) c j", c=NC, p=P))
                nc.vector.tensor_reduce(
                    out=maskf[:],
                    in_=mask_rx[:].rearrange("p (c j) -> p j c", c=NC),
                    axis=X, op=Alu.bitwise_or)
                # idx = mask*8 + q
                nc.vector.tensor_scalar(
                    out=idxu[:], in0=maskf[:], scalar1=3, scalar2=None,
                    op0=Alu.logical_shift_left, op1=Alu.bypass)
                nc.vector.tensor_tensor(out=idxu[:], in0=idxu[:], in1=q[:], op=Alu.add)
                nc.vector.tensor_copy(idx16[:], idxu[:])
                # T2 lookup
                nc.gpsimd.ap_gather(
                    out_ap=t2out[:], in_ap=t2tab[:], idxs_ap=idx16[:],
                    channels=P, num_elems=2048, d=1, num_idxs=T2N)
                # select my lane: AND with m16 then OR-reduce over 16
                nc.vector.tensor_tensor(
                    out=t2out[:].rearrange("p (a b) -> p a b", b=16),
                    in0=t2out[:].rearrange("p (a b) -> p a b", b=16),
                    in1=m16[:, None, :].broadcast_to([P, NPC, 16]),
                    op=Alu.bitwise_and)
                nc.vector.tensor_reduce(
                    out=t2sel[:],
                    in_=t2out[:].rearrange("p (a b) -> p a b", b=16),
                    axis=X, op=Alu.bitwise_or)
                qn = qq.tile([P, NPC], u16, tag="q")
                if it < iters - 1:
                    # bytes + AllGather first (critical path to next iter)
                    nc.vector.tensor_scalar(
                        out=t2tmp[:], in0=t2sel[:], scalar1=16, scalar2=None,
                        op0=Alu.logical_shift_right, op1=Alu.bypass)
                    nc.vector.tensor_copy(bytes16[:], t2tmp[:])
                    ag_in = dram.tile([1, P * NPC], u16, tag="agi")
                    ag_out = dram.tile([1, NC * P * NPC], u16, tag="ago")
                    nc.sync.dma_start(
                        out=ag_in[0:1, :].rearrange("x (p j) -> (x p) j", p=P),
                        in_=bytes16[:])
                    nc.gpsimd.collective_compute(
                        "AllGather", Alu.bypass,
                        replica_groups=[list(range(NC))],
                        ins=[ag_in.opt()], outs=[ag_out.opt()])
                    nc.sync.dma_start(
                        out=data784[:].rearrange("p (c j) -> p c j", c=NC),
                        in_=ag_out[0:1, :].rearrange("x (c p j) -> (x p) c j",
                                                     c=NC, p=P))
                    nc.gpsimd.local_scatter(
                        out_ap=table[:], data_ap=data784[:], idxs_ap=rebuild[:],
                        channels=P, num_elems=NPP, num_idxs=NPP)
                nc.vector.tensor_scalar(
                    out=t2tmp[:], in0=t2sel[:], scalar1=0xFFFF, scalar2=None,
                    op0=Alu.bitwise_and, op1=Alu.bypass)
                nc.vector.tensor_copy(qn[:], t2tmp[:])
                q = qn
            nc.sync.dma_start(out=t_qout[:], in_=q[:])
    nc.compile()
    return nc


def _device_inputs(lay, s0, T):
    state = np.zeros(NTOT, np.int64)
    state[:N_REAL] = np.argmax(np.asarray(s0), axis=1)
    table0 = lay.build_table(state)
    q0 = lay.build_q0(state)
    t2 = lay.build_t2(np.asarray(T))
    m16 = np.zeros((P, 16), np.uint32)
    m16[np.arange(P), np.arange(P) % 16] = 0xFFFFFFFF
    in_maps = []
    for c in range(NC):
        in_maps.append({
            "t_table0": table0,
            "t_q0": q0[c].astype(np.uint16),
            "t_r1idx": lay.r1_idx[c],
            "t_r3idx": lay.r3_idx[c].transpose(1, 0, 2).reshape(P, -1),
            "t_rebuild": lay.rebuild_idx,
            "t_t2": t2,
            "t_m16": m16,
        })
    return in_maps


def kernel(s0, edge_index, T):
    global LAST_EXEC_NS
    from concourse import bass_utils

    s0 = np.asarray(s0)
    edge_index = np.asarray(edge_index)
    Tn = np.asarray(T)
    lay = Layout2(edge_index)
    nc = _build_kernel(lay)
    in_maps = _device_inputs(lay, s0, Tn)
    trace = os.environ.get("BASS_FSM_TRACE", "0") == "1"
    res = bass_utils.run_bass_kernel_spmd(
        nc, in_maps, core_ids=list(range(NC)), trace=trace)
    LAST_EXEC_NS = res.exec_time_ns
    return lay.decode([res.results[c]["t_qout"] for c in range(NC)]).astype(s0.dtype)



# revision 9
# speedup vs baseline: 1.0420x; 1.0420x over previous
"""Self-contained TRN2 Bass kernel for NeuralFSM message passing (v2).

kernel(s0, edge_index, T) -> [100000, 8] float32: 20 FSM iterations on 8
NeuronCores via concourse/bass (SPMD).

Algorithm: per iteration, each edge must deliver byte 1<<state[src] to an
OR-accumulator at its destination. Instead of per-edge ap_gather (27ns/idx,
latency-bound), edges are routed with hardware-scatter primitives:
  expand (DVE broadcast) -> local_scatter into per-dst-partition buckets ->
  one XBAR DMA transpose (cross-partition routing) -> masked local_scatter
  into dst-slot runs -> DVE OR-reduce -> mask AllToAll across cores ->
  FSM lookup via one small ap_gather -> byte AllGather -> table rebuild via
  local_scatter.
Edge->core and edge->block assignments use vectorized Euler splits so every
per-(node,core) and per-(bucket) count is balanced within +-1, keeping all
padding near-optimal.
"""
import os
import sys

import numpy as np

for _p in ("/opt/trn_rl_repo", "/root/.axon_site/_ro/trn_rl_repo", "/root/.axon_site"):
    if os.path.isdir(_p) and _p not in sys.path:
        sys.path.append(_p)


N_REAL = 100000
S = 8
NC = 8
P = 128
NPP = 784
NPC = 98
NTOT = NC * P * NPC  # 100352
ITERS = 20
G = 8
RSUB = 2            # uniform expand sub-run (max node count per (core, block) is 2)


def _cumcount(keys):
    n = len(keys)
    if n == 0:
        return np.zeros(0, np.int64)
    run_start = np.r_[True, keys[1:] != keys[:-1]]
    idx = np.arange(n)
    return idx - np.maximum.accumulate(np.where(run_start, idx, 0))


def _pair_within_runs(order, keys_sorted, size):
    m = np.arange(size)
    pos = _cumcount(keys_sorted)
    cont = np.r_[keys_sorted[1:] == keys_sorted[:-1], False]
    first = (pos % 2 == 0) & cont
    i = np.where(first)[0]
    a, b = order[i], order[i + 1]
    m[a] = b
    m[b] = a
    return m


def euler_color(src, dst):
    """2-color edges: per-src and per-dst counts split into floor/ceil halves."""
    E = len(src)
    sdeg = np.bincount(src)
    ddeg = np.bincount(dst)
    odd_s = np.where(sdeg % 2 == 1)[0]
    odd_d = np.where(ddeg % 2 == 1)[0]
    VS = int(src.max()) + 1
    VD = int(dst.max()) + 1
    ex_src = np.concatenate([src, odd_s, np.full(len(odd_d), VS, np.int64)])
    ex_dst = np.concatenate([dst, np.full(len(odd_s), VD, np.int64), odd_d])
    if len(odd_s) % 2 == 1:
        ex_src = np.r_[ex_src, VS]
        ex_dst = np.r_[ex_dst, VD]
    Etot = len(ex_src)
    o1 = np.argsort(ex_src, kind="stable")
    m1 = _pair_within_runs(o1, ex_src[o1], Etot)
    o2 = np.argsort(ex_dst, kind="stable")
    m2 = _pair_within_runs(o2, ex_dst[o2], Etot)
    g = m2[m1]
    r = np.arange(Etot)
    gg = g.copy()
    for _ in range(30):
        r2 = np.minimum(r, r[gg])
        gg = gg[gg]
        if np.array_equal(r2, r):
            r2 = np.minimum(r2, r2[gg])
            if np.array_equal(r2, r):
                break
        r = r2
    r = np.minimum(r, r[g])
    return (r > r[m1]).astype(np.int8)[:E]


def euler_split8(src, dst):
    E = len(src)
    lab = euler_color(src, dst).astype(np.int64)
    for level in range(1, 3):
        new = np.zeros(E, np.int64)
        for part in range(1 << level):
            sel = np.where(lab == part)[0]
            if len(sel):
                c = euler_color(src[sel], dst[sel])
                new[sel] = (part << 1) | c
        lab = new
    return lab


def _row_classes(vals):
    out = []
    i, n = 0, len(vals)
    while i < n:
        j = i
        while j < n and vals[j] == vals[i]:
            j += 1
        out.append((i, j - i, int(vals[i])))
        i = j
    return out


class Layout2:
    def __init__(self, edge_index):
        src = edge_index[0].astype(np.int64)
        dst = edge_index[1].astype(np.int64)
        E = len(src)

        deg_out = np.bincount(src, minlength=NTOT)
        deg_in = np.bincount(dst, minlength=NTOT)

        order = np.argsort(-deg_out, kind="stable")
        part_of_node = np.empty(NTOT, np.int64)
        part_of_node[order] = np.arange(NTOT) % P

        self.k_src = np.empty(NTOT, np.int64)
        self.k_dst = np.empty(NTOT, np.int64)
        self.node_at_src = np.empty((P, NPP), np.int64)
        self.node_at_dst = np.empty((P, NPP), np.int64)
        for p in range(P):
            nodes = np.where(part_of_node == p)[0]
            o = nodes[np.argsort(-deg_out[nodes], kind="stable")]
            self.k_src[o] = np.arange(NPP)
            self.node_at_src[p] = o
            o2 = nodes[np.argsort(-deg_in[nodes], kind="stable")]
            self.k_dst[o2] = np.arange(NPP)
            self.node_at_dst[p] = o2
        self.part_of_node = part_of_node

        # core split
        ecore = euler_split8(src, dst)
        self.ecore = ecore
        p_src = part_of_node[src]
        p_dst = part_of_node[dst]
        ks = self.k_src[src]
        kd = self.k_dst[dst]

        # block split per (core, p_src): bipartite (node, q)
        lv = (ecore * P + p_src) * NPP + ks        # left vertex id
        rv = (ecore * P + p_src) * P + p_dst       # right vertex id
        eblock = euler_split8(lv, rv)
        self.eblock = eblock

        # expand stream: G blocks x NPP rows x RSUB slots
        # position of edge: block*NPP*RSUB + k_src*RSUB + pos_in (node,core,block) run
        cntg = np.zeros((NC, P, NPP, G), np.int32)
        np.add.at(cntg, (ecore, p_src, ks, eblock), 1)
        assert cntg.max() <= RSUB, cntg.max()
        self.S_exp = G * NPP * RSUB               # 12544

        # reduce runs: L_pad[k] = max over (c,p) of in-count, >=1
        cin = np.zeros((NC, P, NPP), np.int32)
        np.add.at(cin, (ecore, p_dst, kd), 1)
        self.L_pad = np.maximum(1, cin.max(axis=(0, 1))).astype(np.int64)
        self.red_off = np.concatenate([[0], np.cumsum(self.L_pad)])
        self.S_red = int(self.red_off[-1])

        # V windows over k_dst (cumulative L_pad <= 2046)
        self.v_bounds = [0]
        cur = 0
        for k in range(NPP):
            if cur + self.L_pad[k] > 2046:
                self.v_bounds.append(k)
                cur = 0
            cur += self.L_pad[k]
        self.v_bounds.append(NPP)
        self.V = len(self.v_bounds) - 1
        self.v_of_k = np.zeros(NPP, np.int64)
        for v in range(self.V):
            self.v_of_k[self.v_bounds[v]:self.v_bounds[v + 1]] = v
        self.W_v = [int(self.red_off[self.v_bounds[v + 1]] - self.red_off[self.v_bounds[v]])
                    for v in range(self.V)]
        self.W_v_pad = [w + (w % 2) for w in self.W_v]
        self.soff = np.concatenate([[0], np.cumsum(self.W_v_pad)]).astype(np.int64)

        # bucket counts -> B_g
        nb = np.zeros((NC, P, G, P), np.int32)
        np.add.at(nb, (ecore, p_src, eblock, p_dst), 1)
        self.B_g = nb.max(axis=(0, 1, 3)).astype(np.int64)   # [G]
        assert (128 * self.B_g <= 2046).all(), self.B_g
        self.colbase = np.concatenate([[0], np.cumsum(self.B_g)])
        self.C = int(self.colbase[-1])

        # ---- per-edge placement (all cores at once, vectorized) ----
        # expand pos within sub-run
        key_exp = (((ecore * P + p_src) * G + eblock) * NPP + ks)
        o = np.argsort(key_exp, kind="stable")
        sub = _cumcount(key_exp[o])
        t_exp = np.empty(E, np.int64)
        t_exp[o] = (eblock[o] * NPP + ks[o]) * RSUB + sub     # per-partition stream pos

        # R1 bucket fill -> col
        key_b = (((ecore * P + p_src) * G + eblock) * P + p_dst)
        o2 = np.lexsort((t_exp, key_b))
        fill = np.empty(E, np.int64)
        fill[o2] = _cumcount(key_b[o2])
        col = self.colbase[eblock] + fill                     # [E]

        # R3 slot fill
        key_d = (ecore * P + p_dst) * NPP + kd
        o3 = np.lexsort((t_exp, key_d))
        dfill = np.empty(E, np.int64)
        dfill[o3] = _cumcount(key_d[o3])
        slot = self.red_off[kd] + dfill                       # absolute (unpadded-window) slot
        # window-padded absolute slot offset:
        v_e = self.v_of_k[kd]
        wadj = self.soff[v_e] - self.red_off[np.array(self.v_bounds[:-1])][v_e]
        slot_pad = slot + wadj

        # ---- static tensors per core ----
        self.r1_idx = np.full((NC, P, self.S_exp), -1, np.int16)
        seg_len = NPP * RSUB                                  # 1568 per block
        win_rel = (col - self.colbase[eblock]) * P + p_dst
        rel_t = t_exp - eblock * seg_len
        self.r1_idx[ecore, p_src, t_exp] = win_rel.astype(np.int16)

        self.r3_idx = np.full((NC, self.V, P, self.C * P), -1, np.int16)
        tpos = col * P + p_src
        slot_rel = slot_pad - self.soff[v_e]
        self.r3_idx[ecore, v_e, p_dst, tpos] = slot_rel.astype(np.int16)

        # rebuild scatter idx
        self.rebuild_idx = np.empty((P, NPP), np.int16)
        for p in range(P):
            self.rebuild_idx[p] = self.k_src[self.node_at_dst[p]].astype(np.int16)

        # reduce row classes, split at v boundaries, with padded-window offsets
        self.red_classes = []   # (slot_off, k0, ln, L)
        for v in range(self.V):
            k0, k1 = self.v_bounds[v], self.v_bounds[v + 1]
            for (i, ln, L) in _row_classes(self.L_pad[k0:k1]):
                kk = k0 + i
                off = int(self.soff[v] + self.red_off[kk] - self.red_off[k0])
                self.red_classes.append((off, kk, ln, L))

        self.seg_len = seg_len
        self.S_slots = int(self.soff[-1])

    # ---- device input builders ----
    def build_table(self, state):
        tab = np.zeros((P, NPP), np.uint16)
        for p in range(P):
            tab[p] = (1 << state[self.node_at_src[p]]).astype(np.uint16)
        return tab

    def build_q0(self, state):
        q = np.zeros((NC, P, NPC), np.uint32)
        for c in range(NC):
            for p in range(P):
                q[c, p] = state[self.node_at_dst[p, c * NPC:(c + 1) * NPC]]
        return q

    def build_t2(self, T):
        NS = np.argmax(T, axis=2).astype(np.uint32)   # [256, 8]
        tab = np.zeros(2048, np.uint32)
        m = np.repeat(np.arange(256), 8)
        s = np.tile(np.arange(8), 256)
        tab[m * 8 + s] = (np.uint32(1) << NS[m, s]) << 16 | NS[m, s]
        return np.broadcast_to(tab, (P, 2048)).copy()

    def decode(self, qout):
        """qout: [NC][P, NPC] u32 -> one-hot [N_REAL, 8]."""
        st = np.zeros(NTOT, np.int64)
        for c in range(NC):
            for p in range(P):
                st[self.node_at_dst[p, c * NPC:(c + 1) * NPC]] = qout[c][p]
        out = np.zeros((N_REAL, S), np.float32)
        out[np.arange(N_REAL), st[:N_REAL]] = 1.0
        return out



LAST_EXEC_NS = None


def _build_kernel(lay, iters=ITERS):
    from concourse import bacc, tile, mybir

    u16 = mybir.dt.uint16
    u32 = mybir.dt.uint32
    i16 = mybir.dt.int16
    Alu = mybir.AluOpType
    X = mybir.AxisListType.X

    C = lay.C
    CF = C * P                   # transposed stream length per partition
    SEG = lay.seg_len            # 1568
    T2N = 16 * NPC               # 1568 t2 gather idxs per group

    nc = bacc.Bacc("TRN2", target_bir_lowering=False, debug=False,
                   enable_asserts=True, num_devices=NC)
    t_table0 = nc.dram_tensor("t_table0", [P, NPP], u16, kind="ExternalInput")
    t_q0 = nc.dram_tensor("t_q0", [P, NPC], u16, kind="ExternalInput")
    t_r1idx = nc.dram_tensor("t_r1idx", [P, lay.S_exp], i16, kind="ExternalInput")
    t_r3idx = nc.dram_tensor("t_r3idx", [P, lay.V * CF], i16, kind="ExternalInput")
    t_rebuild = nc.dram_tensor("t_rebuild", [P, NPP], i16, kind="ExternalInput")
    t_t2 = nc.dram_tensor("t_t2", [P, 2048], u32, kind="ExternalInput")
    t_m16 = nc.dram_tensor("t_m16", [P, 16], u32, kind="ExternalInput")
    t_di = nc.dram_tensor("t_di", [P, 2], i16, kind="ExternalInput")
    t_qout = nc.dram_tensor("t_qout", [P, NPC], u16, kind="ExternalOutput")

    with tile.TileContext(nc) as tc:
        with tc.tile_pool(name="dram", bufs=2, space="DRAM") as dram, \
             tc.tile_pool(name="per", bufs=1) as per, \
             tc.tile_pool(name="r3p", bufs=3) as r3p, \
             tc.tile_pool(name="qq", bufs=2) as qq:
            table = per.tile([P, NPP], u16)
            r1idx = per.tile([P, lay.S_exp], i16)
            rebuild = per.tile([P, NPP], i16)
            t2tab = per.tile([P, 2048], u32)
            m16 = per.tile([P, 16], u32)
            stream = per.tile([P, lay.S_exp], u16)
            r1out = per.tile([P, CF], u16)
            trout = per.tile([P, CF], u16)
            slots = per.tile([P, lay.S_slots], u16)
            maskp = per.tile([P, NPP], u16)
            mask_rx = per.tile([P, NC * NPC], u16)
            maskf = per.tile([P, NPC], u16)
            idxu = per.tile([P, NPC], u16)
            t2tmp = per.tile([P, NPC], u32)
            idx16 = per.tile([P, NPC], i16)
            t2out = per.tile([P, T2N], u32)
            t2sel = per.tile([P, NPC], u32)
            bytes16 = per.tile([P, NPC], u16)
            data784 = per.tile([P, NPP], u16)
            di = per.tile([P, 2], i16)
            dum_g = per.tile([P, 16], u32)
            dum_s = per.tile([P, 2], u16)

            nc.sync.dma_start(out=table[:], in_=t_table0[:])
            nc.sync.dma_start(out=r1idx[:], in_=t_r1idx[:])
            nc.sync.dma_start(out=rebuild[:], in_=t_rebuild[:])
            nc.sync.dma_start(out=t2tab[:], in_=t_t2[:])
            nc.sync.dma_start(out=m16[:], in_=t_m16[:])
            nc.sync.dma_start(out=di[:], in_=t_di[:])
            q = qq.tile([P, NPC], u16, tag="q")
            nc.sync.dma_start(out=q[:], in_=t_q0[:])

            for it in range(iters):
                # expand: stream[p, (g k r)] = table[p, k]
                for g in range(G):
                    nc.vector.tensor_copy(
                        stream[:, g * SEG:(g + 1) * SEG].rearrange(
                            "p (k r) -> p k r", r=RSUB),
                        table[:, :, None].broadcast_to([P, NPP, RSUB]))
                # R1 + per-window transpose (overlapped)
                for g in range(G):
                    ne = 128 * int(lay.B_g[g])
                    base = lay.colbase[g] * P
                    nc.gpsimd.local_scatter(
                        out_ap=r1out[:, base:base + ne],
                        data_ap=stream[:, g * SEG:(g + 1) * SEG],
                        idxs_ap=r1idx[:, g * SEG:(g + 1) * SEG],
                        channels=P, num_elems=ne, num_idxs=SEG)
                    nc.sync.dma_start(
                        out=trout[:, base:base + ne].rearrange(
                            "p (b q) -> p b q", q=P),
                        in_=r1out[:, base:base + ne], transpose=True)
                # R3 (idx streamed from DRAM, double-buffered)
                for v in range(lay.V):
                    r3i = r3p.tile([P, CF], i16, tag="r3i")
                    nc.sync.dma_start(out=r3i[:], in_=t_r3idx[:, v * CF:(v + 1) * CF])
                    nc.gpsimd.local_scatter(
                        out_ap=slots[:, int(lay.soff[v]):int(lay.soff[v]) + lay.W_v_pad[v]],
                        data_ap=trout[:],
                        idxs_ap=r3i[:],
                        channels=P, num_elems=lay.W_v_pad[v], num_idxs=CF)
                # prefetch the ap_gather library while reduce + AllToAll run:
                # a tiny gather queued right after the last R3 makes the
                # library-load DMA overlap the collective instead of stalling
                # the real T2 gather later.
                nc.gpsimd.ap_gather(
                    out_ap=dum_g[:], in_ap=t2tab[:], idxs_ap=di[:, 0:1],
                    channels=P, num_elems=2048, d=1, num_idxs=16)
                # reduce
                for (off, k0, ln, L) in lay.red_classes:
                    if L == 1:
                        nc.vector.tensor_copy(maskp[:, k0:k0 + ln],
                                              slots[:, off:off + ln])
                    else:
                        nc.vector.tensor_reduce(
                            out=maskp[:, k0:k0 + ln],
                            in_=slots[:, off:off + ln * L].rearrange(
                                "p (a b) -> p a b", b=L),
                            axis=X, op=Alu.bitwise_or)
                # mask exchange (AllToAll)
                a2a_in = dram.tile([1, NTOT], u16, tag="a2ai")
                a2a_out = dram.tile([1, NTOT], u16, tag="a2ao")
                # stage per-peer chunks as their reduce writers finish, so
                # the DRAM staging overlaps the tail R3s/reduces.
                for c in range(NC):
                    nc.sync.dma_start(
                        out=a2a_in[0:1, c * P * NPC:(c + 1) * P * NPC].rearrange(
                            "x (p j) -> (x p) j", p=P),
                        in_=maskp[:, c * NPC:(c + 1) * NPC])
                nc.gpsimd.collective_compute(
                    "AllToAll", Alu.bypass,
                    replica_groups=[list(range(NC))],
                    ins=[a2a_in.opt()], outs=[a2a_out.opt()])
                nc.sync.dma_start(
                    out=mask_rx[:].rearrange("p (c j) -> p c j", c=NC),
                    in_=a2a_out[0:1, :].rearrange("x (c p j) -> (x p) c j", c=NC, p=P))
                nc.vector.tensor_reduce(
                    out=maskf[:],
                    in_=mask_rx[:].rearrange("p (c j) -> p j c", c=NC),
                    axis=X, op=Alu.bitwise_or)
                # idx = mask*8 + q
                nc.vector.tensor_scalar(
                    out=idxu[:], in0=maskf[:], scalar1=3, scalar2=None,
                    op0=Alu.logical_shift_left, op1=Alu.bypass)
                nc.vector.tensor_tensor(out=idxu[:], in0=idxu[:], in1=q[:], op=Alu.add)
                nc.vector.tensor_copy(idx16[:], idxu[:])
                # T2 lookup
                nc.gpsimd.ap_gather(
                    out_ap=t2out[:], in_ap=t2tab[:], idxs_ap=idx16[:],
                    channels=P, num_elems=2048, d=1, num_idxs=T2N)
                # prefetch the local_scatter library under the AllGather so
                # the rebuild scatter doesn't wait for its load.
                nc.gpsimd.local_scatter(
                    out_ap=dum_s[:], data_ap=table[:, 0:2], idxs_ap=di[:, 0:2],
                    channels=P, num_elems=2, num_idxs=2)
                # select my lane: AND with m16 then OR-reduce over 16
                nc.vector.tensor_tensor(
                    out=t2out[:].rearrange("p (a b) -> p a b", b=16),
                    in0=t2out[:].rearrange("p (a b) -> p a b", b=16),
                    in1=m16[:, None, :].broadcast_to([P, NPC, 16]),
                    op=Alu.bitwise_and)
                nc.vector.tensor_reduce(
                    out=t2sel[:],
                    in_=t2out[:].rearrange("p (a b) -> p a b", b=16),
                    axis=X, op=Alu.bitwise_or)
                qn = qq.tile([P, NPC], u16, tag="q")
                if it < iters - 1:
                    # bytes + AllGather first (critical path to next iter)
                    nc.vector.tensor_scalar(
                        out=t2tmp[:], in0=t2sel[:], scalar1=16, scalar2=None,
                        op0=Alu.logical_shift_right, op1=Alu.bypass)
                    nc.vector.tensor_copy(bytes16[:], t2tmp[:])
                    ag_in = dram.tile([1, P * NPC], u16, tag="agi")
                    ag_out = dram.tile([1, NC * P * NPC], u16, tag="ago")
                    nc.sync.dma_start(
                        out=ag_in[0:1, :].rearrange("x (p j) -> (x p) j", p=P),
                        in_=bytes16[:])
                    nc.gpsimd.collective_compute(
                        "AllGather", Alu.bypass,
                        replica_groups=[list(range(NC))],
                        ins=[ag_in.opt()], outs=[ag_out.opt()])
                    nc.sync.dma_start(
                        out=data784[:].rearrange("p (c j) -> p c j", c=NC),
                        in_=ag_out[0:1, :].rearrange("x (c p j) -> (x p) c j",
                                                     c=NC, p=P))
                    nc.gpsimd.local_scatter(
                        out_ap=table[:], data_ap=data784[:], idxs_ap=rebuild[:],
                        channels=P, num_elems=NPP, num_idxs=NPP)
                nc.vector.tensor_scalar(
                    out=t2tmp[:], in0=t2sel[:], scalar1=0xFFFF, scalar2=None,
                    op0=Alu.bitwise_and, op1=Alu.bypass)
                nc.vector.tensor_copy(qn[:], t2tmp[:])
                q = qn
            nc.sync.dma_start(out=t_qout[:], in_=q[:])
    nc.compile()
    return nc


def _device_inputs(lay, s0, T):
    state = np.zeros(NTOT, np.int64)
    state[:N_REAL] = np.argmax(np.asarray(s0), axis=1)
    table0 = lay.build_table(state)
    q0 = lay.build_q0(state)
    t2 = lay.build_t2(np.asarray(T))
    m16 = np.zeros((P, 16), np.uint32)
    m16[np.arange(P), np.arange(P) % 16] = 0xFFFFFFFF
    di = np.broadcast_to(np.array([0, 1], np.int16), (P, 2)).copy()
    in_maps = []
    for c in range(NC):
        in_maps.append({
            "t_table0": table0,
            "t_q0": q0[c].astype(np.uint16),
            "t_r1idx": lay.r1_idx[c],
            "t_r3idx": lay.r3_idx[c].transpose(1, 0, 2).reshape(P, -1),
            "t_rebuild": lay.rebuild_idx,
            "t_t2": t2,
            "t_m16": m16,
            "t_di": di,
        })
    return in_maps


def kernel(s0, edge_index, T):
    global LAST_EXEC_NS
    from concourse import bass_utils

    s0 = np.asarray(s0)
    edge_index = np.asarray(edge_index)
    Tn = np.asarray(T)
    lay = Layout2(edge_index)
    nc = _build_kernel(lay)
    in_maps = _device_inputs(lay, s0, Tn)
    trace = os.environ.get("BASS_FSM_TRACE", "0") == "1"
    res = bass_utils.run_bass_kernel_spmd(
        nc, in_maps, core_ids=list(range(NC)), trace=trace)
    LAST_EXEC_NS = res.exec_time_ns
    return lay.decode([res.results[c]["t_qout"] for c in range(NC)]).astype(s0.dtype)



# revision 17
# speedup vs baseline: 1.2890x; 1.2371x over previous
"""Self-contained TRN2 Bass kernel for NeuralFSM message passing (v2).

kernel(s0, edge_index, T) -> [100000, 8] float32: 20 FSM iterations on 8
NeuronCores via concourse/bass (SPMD).

Algorithm: per iteration, each edge must deliver byte 1<<state[src] to an
OR-accumulator at its destination. Instead of per-edge ap_gather (27ns/idx,
latency-bound), edges are routed with hardware-scatter primitives:
  expand (DVE broadcast) -> local_scatter into per-dst-partition buckets ->
  one XBAR DMA transpose (cross-partition routing) -> masked local_scatter
  into dst-slot runs -> DVE OR-reduce -> mask AllToAll across cores ->
  FSM lookup via one small ap_gather -> byte AllGather -> table rebuild via
  local_scatter.
Edge->core and edge->block assignments use vectorized Euler splits so every
per-(node,core) and per-(bucket) count is balanced within +-1, keeping all
padding near-optimal.
"""
import os
import sys

import numpy as np

for _p in ("/opt/trn_rl_repo", "/root/.axon_site/_ro/trn_rl_repo", "/root/.axon_site"):
    if os.path.isdir(_p) and _p not in sys.path:
        sys.path.append(_p)


N_REAL = 100000
S = 8
NC = 8
P = 128
NPP = 784
NPC = 98
NTOT = NC * P * NPC  # 100352
ITERS = 20
G = 8
RSUB = 2            # uniform expand sub-run (max node count per (core, block) is 2)


def _cumcount(keys):
    n = len(keys)
    if n == 0:
        return np.zeros(0, np.int64)
    run_start = np.r_[True, keys[1:] != keys[:-1]]
    idx = np.arange(n)
    return idx - np.maximum.accumulate(np.where(run_start, idx, 0))


def _pair_within_runs(order, keys_sorted, size):
    m = np.arange(size)
    pos = _cumcount(keys_sorted)
    cont = np.r_[keys_sorted[1:] == keys_sorted[:-1], False]
    first = (pos % 2 == 0) & cont
    i = np.where(first)[0]
    a, b = order[i], order[i + 1]
    m[a] = b
    m[b] = a
    return m


def euler_color(src, dst):
    """2-color edges: per-src and per-dst counts split into floor/ceil halves."""
    E = len(src)
    sdeg = np.bincount(src)
    ddeg = np.bincount(dst)
    odd_s = np.where(sdeg % 2 == 1)[0]
    odd_d = np.where(ddeg % 2 == 1)[0]
    VS = int(src.max()) + 1
    VD = int(dst.max()) + 1
    ex_src = np.concatenate([src, odd_s, np.full(len(odd_d), VS, np.int64)])
    ex_dst = np.concatenate([dst, np.full(len(odd_s), VD, np.int64), odd_d])
    if len(odd_s) % 2 == 1:
        ex_src = np.r_[ex_src, VS]
        ex_dst = np.r_[ex_dst, VD]
    Etot = len(ex_src)
    o1 = np.argsort(ex_src, kind="stable")
    m1 = _pair_within_runs(o1, ex_src[o1], Etot)
    o2 = np.argsort(ex_dst, kind="stable")
    m2 = _pair_within_runs(o2, ex_dst[o2], Etot)
    g = m2[m1]
    r = np.arange(Etot)
    gg = g.copy()
    for _ in range(30):
        r2 = np.minimum(r, r[gg])
        gg = gg[gg]
        if np.array_equal(r2, r):
            r2 = np.minimum(r2, r2[gg])
            if np.array_equal(r2, r):
                break
        r = r2
    r = np.minimum(r, r[g])
    return (r > r[m1]).astype(np.int8)[:E]


def euler_split8(src, dst):
    E = len(src)
    lab = euler_color(src, dst).astype(np.int64)
    for level in range(1, 3):
        new = np.zeros(E, np.int64)
        for part in range(1 << level):
            sel = np.where(lab == part)[0]
            if len(sel):
                c = euler_color(src[sel], dst[sel])
                new[sel] = (part << 1) | c
        lab = new
    return lab


def _row_classes(vals):
    out = []
    i, n = 0, len(vals)
    while i < n:
        j = i
        while j < n and vals[j] == vals[i]:
            j += 1
        out.append((i, j - i, int(vals[i])))
        i = j
    return out


class Layout2:
    def __init__(self, edge_index):
        src = edge_index[0].astype(np.int64)
        dst = edge_index[1].astype(np.int64)
        E = len(src)

        deg_out = np.bincount(src, minlength=NTOT)
        deg_in = np.bincount(dst, minlength=NTOT)

        order = np.argsort(-deg_out, kind="stable")
        part_of_node = np.empty(NTOT, np.int64)
        part_of_node[order] = np.arange(NTOT) % P

        self.k_src = np.empty(NTOT, np.int64)
        self.k_dst = np.empty(NTOT, np.int64)
        self.node_at_src = np.empty((P, NPP), np.int64)
        self.node_at_dst = np.empty((P, NPP), np.int64)
        for p in range(P):
            nodes = np.where(part_of_node == p)[0]
            o = nodes[np.argsort(-deg_out[nodes], kind="stable")]
            self.k_src[o] = np.arange(NPP)
            self.node_at_src[p] = o
            o2 = nodes[np.argsort(-deg_in[nodes], kind="stable")]
            self.k_dst[o2] = np.arange(NPP)
            self.node_at_dst[p] = o2
        self.part_of_node = part_of_node

        # core split
        ecore = euler_split8(src, dst)
        self.ecore = ecore
        p_src = part_of_node[src]
        p_dst = part_of_node[dst]
        ks = self.k_src[src]
        kd = self.k_dst[dst]

        # block split per (core, p_src): bipartite (node, q)
        lv = (ecore * P + p_src) * NPP + ks        # left vertex id
        rv = (ecore * P + p_src) * P + p_dst       # right vertex id
        eblock = euler_split8(lv, rv)
        self.eblock = eblock

        # expand stream: G blocks x NPP rows x RSUB slots
        # position of edge: block*NPP*RSUB + k_src*RSUB + pos_in (node,core,block) run
        cntg = np.zeros((NC, P, NPP, G), np.int32)
        np.add.at(cntg, (ecore, p_src, ks, eblock), 1)
        assert cntg.max() <= RSUB, cntg.max()
        self.S_exp = G * NPP * RSUB               # 12544

        # reduce runs: L_pad[k] = max over (c,p) of in-count, >=1
        cin = np.zeros((NC, P, NPP), np.int32)
        np.add.at(cin, (ecore, p_dst, kd), 1)
        self.L_pad = np.maximum(1, cin.max(axis=(0, 1))).astype(np.int64)
        self.red_off = np.concatenate([[0], np.cumsum(self.L_pad)])
        self.S_red = int(self.red_off[-1])

        # V windows over k_dst (cumulative L_pad <= 2046)
        self.v_bounds = [0]
        cur = 0
        for k in range(NPP):
            if cur + self.L_pad[k] > 2046:
                self.v_bounds.append(k)
                cur = 0
            cur += self.L_pad[k]
        self.v_bounds.append(NPP)
        self.V = len(self.v_bounds) - 1
        self.v_of_k = np.zeros(NPP, np.int64)
        for v in range(self.V):
            self.v_of_k[self.v_bounds[v]:self.v_bounds[v + 1]] = v
        self.W_v = [int(self.red_off[self.v_bounds[v + 1]] - self.red_off[self.v_bounds[v]])
                    for v in range(self.V)]
        self.W_v_pad = [w + (w % 2) for w in self.W_v]
        self.soff = np.concatenate([[0], np.cumsum(self.W_v_pad)]).astype(np.int64)

        # bucket counts -> B_g
        nb = np.zeros((NC, P, G, P), np.int32)
        np.add.at(nb, (ecore, p_src, eblock, p_dst), 1)
        self.B_g = nb.max(axis=(0, 1, 3)).astype(np.int64)   # [G]
        assert (128 * self.B_g <= 2046).all(), self.B_g
        self.colbase = np.concatenate([[0], np.cumsum(self.B_g)])
        self.C = int(self.colbase[-1])

        # ---- per-edge placement (all cores at once, vectorized) ----
        # expand pos within sub-run
        key_exp = (((ecore * P + p_src) * G + eblock) * NPP + ks)
        o = np.argsort(key_exp, kind="stable")
        sub = _cumcount(key_exp[o])
        t_exp = np.empty(E, np.int64)
        t_exp[o] = (eblock[o] * NPP + ks[o]) * RSUB + sub     # per-partition stream pos

        # R1 bucket fill -> col
        key_b = (((ecore * P + p_src) * G + eblock) * P + p_dst)
        o2 = np.lexsort((t_exp, key_b))
        fill = np.empty(E, np.int64)
        fill[o2] = _cumcount(key_b[o2])
        col = self.colbase[eblock] + fill                     # [E]

        # R3 slot fill
        key_d = (ecore * P + p_dst) * NPP + kd
        o3 = np.lexsort((t_exp, key_d))
        dfill = np.empty(E, np.int64)
        dfill[o3] = _cumcount(key_d[o3])
        slot = self.red_off[kd] + dfill                       # absolute (unpadded-window) slot
        # window-padded absolute slot offset:
        v_e = self.v_of_k[kd]
        wadj = self.soff[v_e] - self.red_off[np.array(self.v_bounds[:-1])][v_e]
        slot_pad = slot + wadj

        # ---- static tensors per core ----
        self.r1_idx = np.full((NC, P, self.S_exp), -1, np.int16)
        seg_len = NPP * RSUB                                  # 1568 per block
        win_rel = (col - self.colbase[eblock]) * P + p_dst
        rel_t = t_exp - eblock * seg_len
        self.r1_idx[ecore, p_src, t_exp] = win_rel.astype(np.int16)

        tpos = col * P + p_src
        slot_rel = slot_pad - self.soff[v_e]

        # ---- two-stage R3: per-g window compaction, then per-window scatter
        # stage-A fill rank within (core, p_dst, g, v), ordered by tpos
        keyA = ((ecore * P + p_dst) * G + eblock) * 8 + v_e
        oA = np.lexsort((tpos, keyA))
        fillA = np.empty(E, np.int64)
        fillA[oA] = _cumcount(keyA[oA])
        cntA = np.zeros((NC, P, G, 8), np.int32)
        np.add.at(cntA, (ecore, p_dst, eblock, v_e), 1)
        A_gv = cntA.max(axis=(0, 1))                          # [G, V(8 slots)]
        A_gv = A_gv + (A_gv % 2)                              # even pad
        self.A_gv = A_gv
        # offsets
        self.aoff = np.zeros((G, 9), np.int64)
        self.aoff[:, 1:] = np.cumsum(A_gv, axis=1)
        self.WS_g = self.aoff[:, -1]                          # per-g region size
        self.goff = np.concatenate([[0], np.cumsum(self.WS_g)])
        self.WS = int(self.goff[-1])
        # gsub[v][g] = sum_{g'<g} A_gv[g'][v]
        gsub = np.zeros((8, G), np.int64)
        for v in range(8):
            gsub[v, 1:] = np.cumsum(A_gv[:-1, v])
        self.gsubv = gsub
        self.CAT_v = A_gv.sum(axis=0)                         # [8]
        self.boff = np.concatenate([[0], np.cumsum(self.CAT_v)])
        self.CAT = int(self.boff[-1])

        # stage-A idx stream (indexed by trout position, per g region)
        self.r3a_idx = np.full((NC, P, self.C * P), -1, np.int16)
        outA = self.aoff[eblock, v_e] + fillA                 # rel within g region
        assert outA.max() < 2**15
        self.r3a_idx[ecore, p_dst, tpos] = outA.astype(np.int16)

        # stage-B idx stream (indexed by wcat position)
        self.r3b_idx = np.full((NC, P, self.CAT), -1, np.int16)
        wcpos = self.boff[v_e] + gsub[v_e, eblock] + fillA
        assert slot_rel.max() < 2**15
        self.r3b_idx[ecore, p_dst, wcpos] = slot_rel.astype(np.int16)
        assert (self.WS_g <= 2046).all(), self.WS_g
        assert all(w <= 2046 for w in self.W_v_pad)

        # rebuild scatter idx
        self.rebuild_idx = np.empty((P, NPP), np.int16)
        for p in range(P):
            self.rebuild_idx[p] = self.k_src[self.node_at_dst[p]].astype(np.int16)

        # reduce row classes, split at v boundaries, with padded-window offsets
        self.red_classes = []   # (slot_off, k0, ln, L)
        for v in range(self.V):
            k0, k1 = self.v_bounds[v], self.v_bounds[v + 1]
            for (i, ln, L) in _row_classes(self.L_pad[k0:k1]):
                kk = k0 + i
                off = int(self.soff[v] + self.red_off[kk] - self.red_off[k0])
                self.red_classes.append((off, kk, ln, L))

        self.seg_len = seg_len
        self.S_slots = int(self.soff[-1])

    # ---- device input builders ----
    def build_table(self, state):
        tab = np.zeros((P, NPP), np.uint16)
        for p in range(P):
            tab[p] = (1 << state[self.node_at_src[p]]).astype(np.uint16)
        return tab

    def build_q0(self, state):
        q = np.zeros((NC, P, NPC), np.uint32)
        for c in range(NC):
            for p in range(P):
                q[c, p] = state[self.node_at_dst[p, c * NPC:(c + 1) * NPC]]
        return q

    def build_t2(self, T):
        NS = np.argmax(T, axis=2).astype(np.uint32)   # [256, 8]
        tab = np.zeros(2048, np.uint32)
        m = np.repeat(np.arange(256), 8)
        s = np.tile(np.arange(8), 256)
        tab[m * 8 + s] = (np.uint32(1) << NS[m, s]) << 16 | NS[m, s]
        return np.broadcast_to(tab, (P, 2048)).copy()

    def decode(self, qout):
        """qout: [NC][P, NPC] u32 -> one-hot [N_REAL, 8]."""
        st = np.zeros(NTOT, np.int64)
        for c in range(NC):
            for p in range(P):
                st[self.node_at_dst[p, c * NPC:(c + 1) * NPC]] = qout[c][p]
        out = np.zeros((N_REAL, S), np.float32)
        out[np.arange(N_REAL), st[:N_REAL]] = 1.0
        return out



LAST_EXEC_NS = None


def _build_kernel(lay, iters=ITERS):
    from concourse import bacc, tile, mybir

    u16 = mybir.dt.uint16
    u32 = mybir.dt.uint32
    i16 = mybir.dt.int16
    Alu = mybir.AluOpType
    X = mybir.AxisListType.X

    C = lay.C
    CF = C * P                   # transposed stream length per partition
    SEG = lay.seg_len            # 1568
    T2N = 16 * NPC               # 1568 t2 gather idxs per group

    nc = bacc.Bacc("TRN2", target_bir_lowering=False, debug=False,
                   enable_asserts=True, num_devices=NC)
    t_table0 = nc.dram_tensor("t_table0", [P, NPP], u16, kind="ExternalInput")
    t_q0 = nc.dram_tensor("t_q0", [P, NPC], u16, kind="ExternalInput")
    t_r1idx = nc.dram_tensor("t_r1idx", [P, lay.S_exp], i16, kind="ExternalInput")
    t_r3a = nc.dram_tensor("t_r3a", [P, CF], i16, kind="ExternalInput")
    t_r3b = nc.dram_tensor("t_r3b", [P, lay.CAT], i16, kind="ExternalInput")
    t_rebuild = nc.dram_tensor("t_rebuild", [P, NPP], i16, kind="ExternalInput")
    t_t2 = nc.dram_tensor("t_t2", [P, 2048], u32, kind="ExternalInput")
    t_m16 = nc.dram_tensor("t_m16", [P, 16], u32, kind="ExternalInput")
    t_di = nc.dram_tensor("t_di", [P, 2], i16, kind="ExternalInput")
    t_qout = nc.dram_tensor("t_qout", [P, NPC], u16, kind="ExternalOutput")

    with tile.TileContext(nc) as tc:
        with tc.tile_pool(name="dram", bufs=2, space="DRAM") as dram, \
             tc.tile_pool(name="per", bufs=1) as per, \
             tc.tile_pool(name="qq", bufs=2) as qq:
            table = per.tile([P, NPP], u16)
            r1idx = per.tile([P, lay.S_exp], i16)
            r3a = per.tile([P, CF], i16)
            r3b = per.tile([P, lay.CAT], i16)
            wsort = per.tile([P, lay.WS], u16)
            wcat = per.tile([P, lay.CAT], u16)
            rebuild = per.tile([P, NPP], i16)
            t2tab = per.tile([P, 2048], u32)
            m16 = per.tile([P, 16], u32)
            stream = per.tile([P, lay.S_exp], u16)
            r1out = per.tile([P, CF], u16)
            trout = per.tile([P, CF], u16)
            slots = per.tile([P, lay.S_slots], u16)
            maskp = per.tile([P, NPP], u16)
            mask_rx = per.tile([P, NC * NPC], u16)
            maskf = per.tile([P, NPC], u16)
            idxu = per.tile([P, NPC], u16)
            t2tmp = per.tile([P, NPC], u32)
            idx16 = per.tile([P, NPC], i16)
            t2out = per.tile([P, T2N], u32)
            t2sel = per.tile([P, NPC], u32)
            bytes16 = per.tile([P, NPC], u16)
            data784 = per.tile([P, NPP], u16)
            di = per.tile([P, 2], i16)
            dum_g = per.tile([P, 16], u32)
            dum_s = per.tile([P, 2], u16)

            nc.sync.dma_start(out=table[:], in_=t_table0[:])
            nc.sync.dma_start(out=r1idx[:], in_=t_r1idx[:])
            nc.sync.dma_start(out=r3a[:], in_=t_r3a[:])
            nc.sync.dma_start(out=r3b[:], in_=t_r3b[:])
            nc.sync.dma_start(out=rebuild[:], in_=t_rebuild[:])
            nc.sync.dma_start(out=t2tab[:], in_=t_t2[:])
            nc.sync.dma_start(out=m16[:], in_=t_m16[:])
            nc.sync.dma_start(out=di[:], in_=t_di[:])
            q = qq.tile([P, NPC], u16, tag="q")
            nc.sync.dma_start(out=q[:], in_=t_q0[:])

            for it in range(iters):
                # expand: stream[p, (g k r)] = table[p, k]
                for g in range(G):
                    nc.vector.tensor_copy(
                        stream[:, g * SEG:(g + 1) * SEG].rearrange(
                            "p (k r) -> p k r", r=RSUB),
                        table[:, :, None].broadcast_to([P, NPP, RSUB]))
                # R1 + per-window transpose (overlapped)
                for g in range(G):
                    ne = 128 * int(lay.B_g[g])
                    base = lay.colbase[g] * P
                    nc.gpsimd.local_scatter(
                        out_ap=r1out[:, base:base + ne],
                        data_ap=stream[:, g * SEG:(g + 1) * SEG],
                        idxs_ap=r1idx[:, g * SEG:(g + 1) * SEG],
                        channels=P, num_elems=ne, num_idxs=SEG)
                    nc.sync.dma_start(
                        out=trout[:, base:base + ne].rearrange(
                            "p (b q) -> p b q", q=P),
                        in_=r1out[:, base:base + ne], transpose=True)
                # R3 stage A: per-g window compaction (one scan of trout)
                for g in range(G):
                    ne = 128 * int(lay.B_g[g])
                    base = lay.colbase[g] * P
                    nc.gpsimd.local_scatter(
                        out_ap=wsort[:, int(lay.goff[g]):int(lay.goff[g] + lay.WS_g[g])],
                        data_ap=trout[:, base:base + ne],
                        idxs_ap=r3a[:, base:base + ne],
                        channels=P, num_elems=int(lay.WS_g[g]), num_idxs=ne)
                # regroup [g][v] -> [v][g] (SBUF->SBUF DMA, overlaps stage A)
                for v in range(lay.V):
                    for g in range(G):
                        a = int(lay.A_gv[g, v])
                        if a == 0:
                            continue
                        so = int(lay.goff[g] + lay.aoff[g, v])
                        do = int(lay.boff[v] + lay.gsubv[v, g])
                        nc.sync.dma_start(out=wcat[:, do:do + a],
                                          in_=wsort[:, so:so + a])
                # R3 stage B: per-window scatter from compacted stream
                for v in range(lay.V):
                    cv = int(lay.CAT_v[v])
                    bo = int(lay.boff[v])
                    nc.gpsimd.local_scatter(
                        out_ap=slots[:, int(lay.soff[v]):int(lay.soff[v]) + lay.W_v_pad[v]],
                        data_ap=wcat[:, bo:bo + cv],
                        idxs_ap=r3b[:, bo:bo + cv],
                        channels=P, num_elems=lay.W_v_pad[v], num_idxs=cv)
                # prefetch the ap_gather library while reduce + AllToAll run:
                # a tiny gather queued right after the last R3 makes the
                # library-load DMA overlap the collective instead of stalling
                # the real T2 gather later.
                nc.gpsimd.ap_gather(
                    out_ap=dum_g[:], in_ap=t2tab[:], idxs_ap=di[:, 0:1],
                    channels=P, num_elems=2048, d=1, num_idxs=16)
                # reduce
                for (off, k0, ln, L) in lay.red_classes:
                    if L == 1:
                        nc.vector.tensor_copy(maskp[:, k0:k0 + ln],
                                              slots[:, off:off + ln])
                    else:
                        nc.vector.tensor_reduce(
                            out=maskp[:, k0:k0 + ln],
                            in_=slots[:, off:off + ln * L].rearrange(
                                "p (a b) -> p a b", b=L),
                            axis=X, op=Alu.bitwise_or)
                # mask exchange (AllToAll)
                a2a_in = dram.tile([1, NTOT], u16, tag="a2ai")
                a2a_out = dram.tile([1, NTOT], u16, tag="a2ao")
                # stage per-peer chunks as their reduce writers finish, so
                # the DRAM staging overlaps the tail R3s/reduces.
                for c in range(NC):
                    nc.sync.dma_start(
                        out=a2a_in[0:1, c * P * NPC:(c + 1) * P * NPC].rearrange(
                            "x (p j) -> (x p) j", p=P),
                        in_=maskp[:, c * NPC:(c + 1) * NPC])
                nc.gpsimd.collective_compute(
                    "AllToAll", Alu.bypass,
                    replica_groups=[list(range(NC))],
                    ins=[a2a_in.opt()], outs=[a2a_out.opt()])
                nc.sync.dma_start(
                    out=mask_rx[:].rearrange("p (c j) -> p c j", c=NC),
                    in_=a2a_out[0:1, :].rearrange("x (c p j) -> (x p) c j", c=NC, p=P))
                nc.vector.tensor_reduce(
                    out=maskf[:],
                    in_=mask_rx[:].rearrange("p (c j) -> p j c", c=NC),
                    axis=X, op=Alu.bitwise_or)
                # idx = mask*8 + q
                nc.vector.tensor_scalar(
                    out=idxu[:], in0=maskf[:], scalar1=3, scalar2=None,
                    op0=Alu.logical_shift_left, op1=Alu.bypass)
                nc.vector.tensor_tensor(out=idxu[:], in0=idxu[:], in1=q[:], op=Alu.add)
                nc.vector.tensor_copy(idx16[:], idxu[:])
                # T2 lookup
                nc.gpsimd.ap_gather(
                    out_ap=t2out[:], in_ap=t2tab[:], idxs_ap=idx16[:],
                    channels=P, num_elems=2048, d=1, num_idxs=T2N)
                # prefetch the local_scatter library under the AllGather so
                # the rebuild scatter doesn't wait for its load.
                nc.gpsimd.local_scatter(
                    out_ap=dum_s[:], data_ap=table[:, 0:2], idxs_ap=di[:, 0:2],
                    channels=P, num_elems=2, num_idxs=2)
                # select my lane: AND with m16 then OR-reduce over 16
                nc.vector.tensor_tensor(
                    out=t2out[:].rearrange("p (a b) -> p a b", b=16),
                    in0=t2out[:].rearrange("p (a b) -> p a b", b=16),
                    in1=m16[:, None, :].broadcast_to([P, NPC, 16]),
                    op=Alu.bitwise_and)
                nc.vector.tensor_reduce(
                    out=t2sel[:],
                    in_=t2out[:].rearrange("p (a b) -> p a b", b=16),
                    axis=X, op=Alu.bitwise_or)
                qn = qq.tile([P, NPC], u16, tag="q")
                if it < iters - 1:
                    # bytes + AllGather first (critical path to next iter)
                    nc.vector.tensor_scalar(
                        out=t2tmp[:], in0=t2sel[:], scalar1=16, scalar2=None,
                        op0=Alu.logical_shift_right, op1=Alu.bypass)
                    nc.vector.tensor_copy(bytes16[:], t2tmp[:])
                    ag_in = dram.tile([1, P * NPC], u16, tag="agi")
                    ag_out = dram.tile([1, NC * P * NPC], u16, tag="ago")
                    nc.sync.dma_start(
                        out=ag_in[0:1, :].rearrange("x (p j) -> (x p) j", p=P),
                        in_=bytes16[:])
                    nc.gpsimd.collective_compute(
                        "AllGather", Alu.bypass,
                        replica_groups=[list(range(NC))],
                        ins=[ag_in.opt()], outs=[ag_out.opt()])
                    nc.sync.dma_start(
                        out=data784[:].rearrange("p (c j) -> p c j", c=NC),
                        in_=ag_out[0:1, :].rearrange("x (c p j) -> (x p) c j",
                                                     c=NC, p=P))
                    nc.gpsimd.local_scatter(
                        out_ap=table[:], data_ap=data784[:], idxs_ap=rebuild[:],
                        channels=P, num_elems=NPP, num_idxs=NPP)
                nc.vector.tensor_scalar(
                    out=t2tmp[:], in0=t2sel[:], scalar1=0xFFFF, scalar2=None,
                    op0=Alu.bitwise_and, op1=Alu.bypass)
                nc.vector.tensor_copy(qn[:], t2tmp[:])
                q = qn
            nc.sync.dma_start(out=t_qout[:], in_=q[:])
    nc.compile()
    return nc


def _device_inputs(lay, s0, T):
    state = np.zeros(NTOT, np.int64)
    state[:N_REAL] = np.argmax(np.asarray(s0), axis=1)
    table0 = lay.build_table(state)
    q0 = lay.build_q0(state)
    t2 = lay.build_t2(np.asarray(T))
    m16 = np.zeros((P, 16), np.uint32)
    m16[np.arange(P), np.arange(P) % 16] = 0xFFFFFFFF
    di = np.broadcast_to(np.array([0, 1], np.int16), (P, 2)).copy()
    in_maps = []
    for c in range(NC):
        in_maps.append({
            "t_table0": table0,
            "t_q0": q0[c].astype(np.uint16),
            "t_r1idx": lay.r1_idx[c],
            "t_r3a": lay.r3a_idx[c],
            "t_r3b": lay.r3b_idx[c],
            "t_rebuild": lay.rebuild_idx,
            "t_t2": t2,
            "t_m16": m16,
            "t_di": di,
        })
    return in_maps


def kernel(s0, edge_index, T):
    global LAST_EXEC_NS
    from concourse import bass_utils

    s0 = np.asarray(s0)
    edge_index = np.asarray(edge_index)
    Tn = np.asarray(T)
    lay = Layout2(edge_index)
    nc = _build_kernel(lay)
    in_maps = _device_inputs(lay, s0, Tn)
    trace = os.environ.get("BASS_FSM_TRACE", "0") == "1"
    res = bass_utils.run_bass_kernel_spmd(
        nc, in_maps, core_ids=list(range(NC)), trace=trace)
    LAST_EXEC_NS = res.exec_time_ns
    return lay.decode([res.results[c]["t_qout"] for c in range(NC)]).astype(s0.dtype)

